# revision 1
# baseline (speedup 1.0000x reference)
"""E3CoordLayer GNN message-passing kernel for 8 Trainium2 NeuronCores.

Strategy (edge-parallel, row-range sharded):
  - Sort edges by row; core c owns rows [c*6250, (c+1)*6250).
  - Within a core, edges are grouped into 49 node-blocks of 128 rows; within a
    block, split by col parity into 2 runs; each run padded to T_P tiles of 128
    edges (T_P = global max, uniform for SPMD).
  - h is shipped as bf16 "pair" rows [25024, 256] (=h.reshape) so the col
    gather uses 512B descriptors and int16 indices (col>>1 < 25024); the col
    parity selects the plane of the transposed gather output at zero cost.
  - h[row] is never gathered: q = h @ W1a is computed on-device at node level
    per 128-row block; the per-edge expansion q[row_e] is fused into the z1
    matmul via M[n,e] built 512-wide (PE K=1 broadcast of relrow + DVE
    is_equal against a channel iota).
  - MLP runs feature-major: z1[h1,e], z2[h2,e]; z3 edge-major via per-tile
    matmul (lhsT=z2 tile, rhs=W3) -> tanh per run -> scale[e] per partition.
  - Aggregation: one wide DVE is_equal builds onehot[e,(t,n)] per run; cd is
    scaled by tanh in one wide DVE op; agg[3,n] += cd_sc[e,3]^T @ onehot
    accumulates in psum per block; then (agg + x^T) * flags^T -> out f32.
  - Gathers carry an explicit cross-run dependency guard so consumers never
    race the xbar-transposed DMA writes (intermittent HW corruption without
    it).
  - Output: concat core outputs, transpose, trim to [50000, 3].
"""
import sys
import os

sys.path.insert(0, "/opt/trn_rl_repo")

import numpy as np
import ml_dtypes

N_NODES = 50000
N_EDGES = 800000
HIDDEN = 128
EDGE_DIM = 16
COORDS_RANGE = 15.0
NCORES = 8
P = 128
NPC = N_NODES // NCORES          # 6250 nodes per core
NB = (NPC + P - 1) // P          # 49 blocks per core
NPAD = NB * P                    # 6272 padded nodes per core
NPAIR = (N_NODES + 1) // 2 + 12  # 25012 -> pad a bit; see below

_BF16 = ml_dtypes.bfloat16


def _wrap_idx(idx_call):
    """Wrap a call's int16 index list [NI] -> [128, NI//16] (16-part wrap,
    replicated 8x down partitions)."""
    ni = idx_call.shape[0]
    w = idx_call.reshape(ni // 16, 16).T  # [16, NI//16]
    return np.tile(w, (8, 1))             # [128, NI//16]


def _build_nc(TP, half_nis):
    import concourse.bass as bass
    import concourse.mybir as mybir
    import concourse.tile as tile
    from concourse import bacc
    from concourse import library_config

    dt = mybir.dt
    S = NB * 2 * TP * P              # edge slots per core
    NT = NB * 2 * TP                 # tiles per core
    RUNW = TP * P                    # edges per run
    NPAIRT = N_NODES // 2 + P        # pair-table rows (25128), idx < 25000+ ok

    nc = bacc.Bacc("TRN2", target_bir_lowering=False, debug=False,
                   num_devices=NCORES, num_swdge_queues=4,
                   dynamic_dma_scratch_size=65536)

    hp = nc.dram_tensor("hp", [NPAIRT, 2 * HIDDEN], dt.bfloat16, kind="ExternalInput")
    hTs = nc.dram_tensor("hTs", [P, NPAD], dt.bfloat16, kind="ExternalInput")
    idxw = nc.dram_tensor("idxw", [P, S // 16], dt.int16, kind="ExternalInput")
    relrow = nc.dram_tensor("relrow", [P, NT], dt.bfloat16, kind="ExternalInput")
    relrowT = nc.dram_tensor("relrowT", [1, S], dt.bfloat16, kind="ExternalInput")
    eaT = nc.dram_tensor("eaT", [EDGE_DIM + 1, S], dt.bfloat16, kind="ExternalInput")
    cdsc = nc.dram_tensor("cdsc", [S, 3], dt.bfloat16, kind="ExternalInput")
    xT3 = nc.dram_tensor("xT3", [3, NPAD], dt.float32, kind="ExternalInput")
    flg3 = nc.dram_tensor("flg3", [3, NPAD], dt.float32, kind="ExternalInput")
    w1a = nc.dram_tensor("w1a", [HIDDEN, HIDDEN], dt.bfloat16, kind="ExternalInput")
    w1b = nc.dram_tensor("w1b", [HIDDEN, HIDDEN], dt.bfloat16, kind="ExternalInput")
    w1c = nc.dram_tensor("w1c", [EDGE_DIM + 1, HIDDEN], dt.bfloat16, kind="ExternalInput")
    w2 = nc.dram_tensor("w2", [HIDDEN, HIDDEN], dt.bfloat16, kind="ExternalInput")
    w3 = nc.dram_tensor("w3", [HIDDEN, 1], dt.bfloat16, kind="ExternalInput")
    b2 = nc.dram_tensor("b2", [HIDDEN, 1], dt.float32, kind="ExternalInput")
    outT = nc.dram_tensor("outT", [3, NPAD], dt.float32, kind="ExternalOutput")

    AF = mybir.ActivationFunctionType
    ALU = mybir.AluOpType

    with tile.TileContext(nc) as tc:
        nc.gpsimd.load_library(library_config.mlp)
        tc.strict_bb_all_engine_barrier()
        with (
            tc.tile_pool(name="const", bufs=1) as cp,
            tc.tile_pool(name="gath", bufs=4) as gp,
            tc.tile_pool(name="work", bufs=2) as wp,
            tc.tile_pool(name="oh", bufs=4) as ohp,
            tc.tile_pool(name="scp", bufs=3) as scp,
            tc.tile_pool(name="pbig", bufs=1, space="PSUM") as pbig,
            tc.tile_pool(name="psmall", bufs=3, space="PSUM") as psmall,
            tc.tile_pool(name="pagg", bufs=2, space="PSUM") as pagg,
        ):
            # ---- resident constants
            w1a_sb = cp.tile([HIDDEN, HIDDEN], dt.bfloat16)
            nc.sync.dma_start(out=w1a_sb[:], in_=w1a[:])
            w1b_sb = cp.tile([HIDDEN, HIDDEN], dt.bfloat16)
            nc.sync.dma_start(out=w1b_sb[:], in_=w1b[:])
            w1c_sb = cp.tile([EDGE_DIM + 1, HIDDEN], dt.bfloat16)
            nc.sync.dma_start(out=w1c_sb[:], in_=w1c[:])
            w2_sb = cp.tile([HIDDEN, HIDDEN], dt.bfloat16)
            nc.sync.dma_start(out=w2_sb[:], in_=w2[:])
            w3_sb = cp.tile([HIDDEN, 1], dt.bfloat16)
            nc.sync.dma_start(out=w3_sb[:], in_=w3[:])
            b2_sb = cp.tile([HIDDEN, 1], dt.float32)
            nc.sync.dma_start(out=b2_sb[:], in_=b2[:])
            idx_sb = cp.tile([P, S // 16], dt.int16)
            nc.sync.dma_start(out=idx_sb[:], in_=idxw[:])
            rel_sb = cp.tile([P, NT], dt.bfloat16)
            nc.sync.dma_start(out=rel_sb[:], in_=relrow[:])
            ones_sb = cp.tile([1, P], dt.bfloat16)
            nc.vector.memset(ones_sb[:], 1.0)
            x_sb = cp.tile([3, NPAD], dt.float32)
            nc.sync.dma_start(out=x_sb[:], in_=xT3[:])
            f_sb = cp.tile([3, NPAD], dt.float32)
            nc.sync.dma_start(out=f_sb[:], in_=flg3[:])
            ident = cp.tile([P, P], dt.bfloat16)
            from concourse.masks import make_identity
            make_identity(nc, ident[:])
            iota_i = cp.tile([P, P], dt.int32)
            nc.gpsimd.iota(iota_i[:], pattern=[[1, P]], base=0, channel_multiplier=0)
            iota16 = cp.tile([P, P], dt.bfloat16)
            nc.vector.tensor_copy(out=iota16[:], in_=iota_i[:])
            iota_big = cp.tile([P, TP * P], dt.bfloat16)
            for t in range(TP):
                nc.vector.tensor_copy(out=iota_big[:, t * P:(t + 1) * P],
                                      in_=iota16[:])
            chio_i = cp.tile([P, 1], dt.int32)
            nc.gpsimd.iota(chio_i[:], pattern=[[1, 1]], base=0, channel_multiplier=1)
            chio = cp.tile([P, 1], dt.float32)
            nc.vector.tensor_copy(out=chio[:], in_=chio_i[:])

            # ---- q = h @ W1a per node block (node-major in SBUF)
            hTs_sb = cp.tile([P, NPAD], dt.bfloat16)
            nc.sync.dma_start(out=hTs_sb[:], in_=hTs[:])
            q_sb = cp.tile([P, NB, HIDDEN], dt.bfloat16)
            for b in range(NB):
                qp = psmall.tile([HIDDEN, P], dt.float32, tag="ps")
                nc.tensor.matmul(qp[:], lhsT=w1a_sb[:], rhs=hTs_sb[:, b * P:(b + 1) * P],
                                 start=True, stop=True)
                qT = wp.tile([HIDDEN, P], dt.bfloat16, tag="qT")
                nc.vector.tensor_copy(out=qT[:], in_=qp[:])
                qp2 = psmall.tile([P, HIDDEN], dt.bfloat16, tag="ps")
                nc.tensor.transpose(out=qp2[:], in_=qT[:], identity=ident[:])
                nc.vector.tensor_copy(out=q_sb[:, b, :], in_=qp2[:])
            tc.strict_bb_all_engine_barrier()

            # ---- main loop
            from concourse.bass import _add_dep_helper
            z1b_by_run = {}
            for b in range(NB):
                cd_sb = gp.tile([P, 2 * TP, 3], dt.bfloat16, tag="cd")
                nc.sync.dma_start(
                    out=cd_sb[:],
                    in_=cdsc[b * 2 * TP * P:(b + 1) * 2 * TP * P, :].rearrange(
                        "(t p) c -> p t c", p=P))
                aggp = pagg.tile([3, P], dt.float32, tag="agg")
                for r in range(2):
                    run = b * 2 + r
                    g0 = run * TP          # first global tile of run
                    e0 = g0 * P            # first slot
                    # col-pair gathers for the run (one tile per call for
                    # contiguous out APs)
                    relT_sb = gp.tile([1, RUNW], dt.bfloat16, tag="relT")
                    nc.sync.dma_start(out=relT_sb[:], in_=relrowT[:, e0:e0 + RUNW])
                    pairs = []
                    pair_t0 = []
                    off = 0
                    for ci, ni in enumerate(half_nis):
                        pr = gp.tile([P, 2, ni], dt.bfloat16, tag=f"pair{ci}")
                        gi = nc.gpsimd.dma_gather(
                            pr[:], hp[:],
                            idx_sb[:, (e0 + off) // 16:(e0 + off + ni) // 16],
                            ni, ni, 2 * HIDDEN, transpose=True,
                            queue_num=(run * 2 + ci) % 4,
                        )
                        # xbar-flush guard: consumers of the gather issued two
                        # runs earlier must wait until this gather retired on
                        # Q7, giving that DMA time to fully land.
                        for prev in z1b_by_run.get(run - 2, ()):
                            _add_dep_helper(prev, gi.ins,
                                            reason="gather xbar-flush guard")
                        pairs.append(pr)
                        pair_t0.append(off // P)
                        off += ni
                    # eaT slice for the run
                    ea_sb = gp.tile([EDGE_DIM + 1, RUNW], dt.bfloat16, tag="ea")
                    nc.sync.dma_start(out=ea_sb[:], in_=eaT[:, e0:e0 + RUNW])

                    oh_big = ohp.tile([P, TP * P], dt.bfloat16, tag="oh")
                    nc.vector.tensor_tensor(
                        out=oh_big[:], in0=iota_big[:],
                        in1=rel_sb[:, g0:g0 + TP].to_broadcast([P, TP, P]),
                        op=ALU.is_equal)
                    z1p = pbig.tile([P, RUNW], dt.float32, tag="zp")
                    # z1-B: W1b^T @ hcolT, batched N<=512 per gather half
                    z1b_list = []
                    for ci, ni in enumerate(half_nis):
                        base = pair_t0[ci] * P
                        for c0 in range(0, ni, 512):
                            cw = min(512, ni - c0)
                            mm = nc.tensor.matmul(
                                z1p[:, base + c0:base + c0 + cw], lhsT=w1b_sb[:],
                                rhs=pairs[ci][:, r, c0:c0 + cw],
                                start=True, stop=False)
                            z1b_list.append(mm.ins)
                    z1b_by_run[run] = z1b_list
                    # z1-C: W1c'^T @ eaT, batched
                    for c0 in range(0, RUNW, 512):
                        cw = min(512, RUNW - c0)
                        nc.tensor.matmul(z1p[:, c0:c0 + cw], lhsT=w1c_sb[:],
                                         rhs=ea_sb[:, c0:c0 + cw],
                                         start=False, stop=False)
                    # M[n, e] built 512-wide: PE bcast of relrow + is_equal
                    # against channel iota; z1-A batched N<=512
                    for c0 in range(0, RUNW, 512):
                        cw = min(512, RUNW - c0)
                        bc = psmall.tile([P, 512], dt.float32, tag="ps")
                        nc.tensor.matmul(bc[:, :cw], lhsT=ones_sb[:],
                                         rhs=relT_sb[:, c0:c0 + cw],
                                         start=True, stop=True)
                        m_sb = wp.tile([P, 512], dt.bfloat16, tag="m")
                        nc.vector.tensor_scalar(
                            out=m_sb[:, :cw], in0=bc[:, :cw], scalar1=chio[:],
                            scalar2=None, op0=ALU.is_equal)
                        nc.tensor.matmul(z1p[:, c0:c0 + cw], lhsT=q_sb[:, b, :],
                                         rhs=m_sb[:, :cw], start=False, stop=True)

                    z1sb = wp.tile([P, RUNW], dt.bfloat16, tag="z1")
                    nc.scalar.activation(out=z1sb[:], in_=z1p[:], func=AF.Silu)
                    z2p = pbig.tile([P, RUNW], dt.float32, tag="zp")
                    for c0 in range(0, RUNW, 512):
                        cw = min(512, RUNW - c0)
                        nc.tensor.matmul(z2p[:, c0:c0 + cw], lhsT=w2_sb[:],
                                         rhs=z1sb[:, c0:c0 + cw], start=True, stop=True)
                    z2sb = wp.tile([P, RUNW], dt.bfloat16, tag="z2")
                    nc.scalar.activation(out=z2sb[:], in_=z2p[:], func=AF.Silu,
                                         bias=b2_sb[:])
                    z3p = pagg.tile([P, TP], dt.float32, tag="agg")
                    for t in range(TP):
                        el = t * P
                        nc.tensor.matmul(z3p[:, t:t + 1], lhsT=z2sb[:, el:el + P],
                                         rhs=w3_sb[:], start=True, stop=True)
                    sc = scp.tile([P, TP], dt.bfloat16, tag="sc")
                    nc.scalar.activation(out=sc[:], in_=z3p[:], func=AF.Tanh)
                    cdt_big = wp.tile([P, TP, 3], dt.bfloat16, tag="cdt")
                    nc.vector.tensor_tensor(
                        out=cdt_big[:], in0=cd_sb[:, r * TP:(r + 1) * TP, :],
                        in1=sc[:].to_broadcast([P, TP, 3]), op=ALU.mult)
                    for t in range(TP):
                        k = r * TP + t
                        nc.tensor.matmul(aggp[:], lhsT=cdt_big[:, t, :],
                                         rhs=oh_big[:, t * P:(t + 1) * P],
                                         start=(k == 0), stop=(k == 2 * TP - 1))

                osb = wp.tile([3, P], dt.float32, tag="osb")
                nc.vector.tensor_tensor(out=osb[:], in0=aggp[:],
                                        in1=x_sb[:, b * P:(b + 1) * P], op=ALU.add)
                nc.vector.tensor_tensor(out=osb[:], in0=osb[:],
                                        in1=f_sb[:, b * P:(b + 1) * P], op=ALU.mult)
                nc.sync.dma_start(out=outT[:, b * P:(b + 1) * P], in_=osb[:])
    nc.compile()
    return nc


def _host_prep(h, x, edge_index, edge_attr, coord_diff, flags):
    """Sort/group/pad edges; build per-core input maps. Returns (in_maps, TP, half_nis)."""
    row = np.asarray(edge_index[0], dtype=np.int64)
    col = np.asarray(edge_index[1], dtype=np.int64)
    E = row.shape[0]

    core = row // NPC                      # 0..7
    blk = (row % NPC) // P                 # 0..48
    par = col & 1
    # group key: (core, blk, parity); stable order within groups irrelevant
    key = (core * NB + blk) * 2 + par
    order = np.argsort(key, kind="stable")
    ksort = key[order]
    # counts per (core, blk, par)
    ngroups = NCORES * NB * 2
    counts = np.bincount(ksort, minlength=ngroups)
    TP = int((counts.max() + P - 1) // P)
    TP = max(TP, 1)
    RUNW = TP * P
    S = NB * 2 * RUNW
    # gather calls per run: Q7 ring is 128 entries/core -> <1024 idx per
    # call. Call boundaries must land on 512-col multiples so the z1-B
    # matmul chunks never cross a PSUM bank.
    k, rem = RUNW // 512, RUNW % 512
    if k == 0:
        half_nis = [RUNW]
    elif 512 + rem <= 896:
        half_nis = [512] * (k - 1) + [512 + rem]
    else:
        half_nis = [512] * k + ([rem] if rem else [])

    # slot assignment: group g occupies slots [g_local * RUNW ...) on its core
    gstart = np.zeros(ngroups + 1, dtype=np.int64)
    gstart[1:] = np.cumsum(counts)
    # position of each sorted edge within its group
    within = np.arange(E, dtype=np.int64) - gstart[ksort]
    glocal = ksort % (NB * 2)
    slot = glocal * RUNW + within          # slot on the core
    ecore = ksort // (NB * 2)

    h_bf = np.ascontiguousarray(h.astype(_BF16))
    NPAIRT = N_NODES // 2 + P
    hp = np.zeros((NPAIRT, 2 * HIDDEN), dtype=_BF16)
    hp[:N_NODES // 2] = h_bf.reshape(N_NODES // 2, 2 * HIDDEN)
    hT = np.ascontiguousarray(h_bf.T)      # [128, N]

    ea = edge_attr.astype(np.float32)
    cd15 = (coord_diff.astype(np.float32) * COORDS_RANGE).astype(_BF16)

    in_maps = []
    for c in range(NCORES):
        m = ecore == c
        sl = slot[m]
        eidx = order[m]
        # per-slot arrays (pad slots stay 0)
        colw = np.zeros(S, dtype=np.int16)
        colw[sl] = (col[eidx] >> 1).astype(np.int16)
        rel = np.zeros(S, dtype=np.int16)
        rel[sl] = ((row[eidx] % NPC) % P).astype(np.int16)
        eaT = np.zeros((EDGE_DIM + 1, S), dtype=_BF16)
        eaT[:EDGE_DIM, sl] = ea[eidx].T.astype(_BF16)
        eaT[EDGE_DIM, sl] = np.float32(1.0)
        cds = np.zeros((S, 3), dtype=_BF16)
        cds[sl] = cd15[eidx]

        # wrap indices per gather call
        idxw = np.zeros((P, S // 16), dtype=np.int16)
        coff = 0
        for g in range(NB * 2):
            base = g * RUNW
            for ni in half_nis:
                idxw[:, coff:coff + ni // 16] = _wrap_idx(colw[base:base + ni])
                base += ni
                coff += ni // 16
        relw = rel.reshape(S // P, P).T.astype(np.float32).astype(_BF16)  # [128, NT]
        relT = rel.astype(np.float32).astype(_BF16).reshape(1, S)

        n0 = c * NPC
        hTs = np.zeros((P, NPAD), dtype=_BF16)
        hTs[:, :NPC] = hT[:, n0:n0 + NPC]
        xT3 = np.zeros((3, NPAD), dtype=np.float32)
        xT3[:, :NPC] = x[n0:n0 + NPC].T.astype(np.float32)
        flg3 = np.zeros((3, NPAD), dtype=np.float32)
        flg3[:, :NPC] = np.broadcast_to(
            flags[n0:n0 + NPC].astype(np.float32).T, (3, NPC))

        in_maps.append({
            "hp": hp, "hTs": hTs, "idxw": idxw, "relrow": relw, "relrowT": relT,
            "eaT": np.ascontiguousarray(eaT), "cdsc": cds,
            "xT3": xT3, "flg3": flg3,
        })
    return in_maps, TP, half_nis


def kernel(h, x, edge_index, edge_attr, coord_diff, flags, edge_mask,
           W1, b1, W2, b2, W3):
    from concourse.bass_utils import run_bass_kernel_spmd

    h = np.asarray(h, dtype=np.float32)
    x = np.asarray(x, dtype=np.float32)
    in_maps, TP, half_nis = _host_prep(
        h, x, np.asarray(edge_index), np.asarray(edge_attr),
        np.asarray(coord_diff), np.asarray(flags))

    # weights (shared across cores)
    W1 = np.asarray(W1, dtype=np.float32)
    w1a = np.ascontiguousarray(W1[:HIDDEN].astype(_BF16))
    w1b = np.ascontiguousarray(W1[HIDDEN:2 * HIDDEN].astype(_BF16))
    w1c = np.zeros((EDGE_DIM + 1, HIDDEN), dtype=_BF16)
    w1c[:EDGE_DIM] = W1[2 * HIDDEN:].astype(_BF16)
    w1c[EDGE_DIM] = np.asarray(b1, dtype=np.float32).astype(_BF16)
    wshare = {
        "w1a": w1a, "w1b": w1b, "w1c": w1c,
        "w2": np.ascontiguousarray(np.asarray(W2, np.float32).astype(_BF16)),
        "w3": np.ascontiguousarray(np.asarray(W3, np.float32).astype(_BF16)),
        "b2": np.asarray(b2, np.float32).reshape(HIDDEN, 1),
    }
    for m in in_maps:
        m.update(wshare)

    nc = _build_nc(TP, half_nis)
    res = run_bass_kernel_spmd(nc, in_maps, core_ids=list(range(NCORES)),
                               trace=os.environ.get("BASS_TRACE") == "1")
    global last_result
    last_result = res
    out = np.empty((N_NODES, 3), dtype=np.float32)
    for c in range(NCORES):
        out[c * NPC:(c + 1) * NPC] = res.results[c]["outT"][:, :NPC].T
    return out


last_result = None



# revision 5
# speedup vs baseline: 1.5833x; 1.5833x over previous
"""E3CoordLayer GNN message-passing kernel for 8 Trainium2 NeuronCores.

Strategy (edge-parallel, row-range sharded, v2):
  - Sort edges by row; core c owns rows [c*6250, (c+1)*6250). Rows grouped
    into NB blocks of BLKR=112 rows; within a block, edges split into 2 runs
    by col range (col < 25000 -> lo table, else hi table) so the h gather
    uses 256B single-row descriptors with int16 indices.
  - Each run is padded to RUNW = TP*128 slots (TP = global max, uniform for
    SPMD). Gathers use single_packet=False so descriptors drain across all
    16 SDMA engines instead of one engine per call.
  - h[row] is never gathered: q = h @ W1a computed per node block on device;
    the per-edge expansion q[row_e] uses a host-shipped fp8 one-hot
    M[rel, slot] as the matmul rhs (no on-device M build).
  - The aggregation one-hot oh[slot%128, (t, rel)] is also host-shipped fp8.
  - MLP runs feature-major: z1[h1,e] accumulates W1b^T hcol + W1c'^T ea
    (b1 folded via a ones-row in eaT) + q^T M; silu; z2 = W2^T z1sb; silu
    with b2 bias; z3 per tile via lhsT=z2-tile, rhs=w3 -> z3p[p, r*TP+t];
    tanh once per block; cdt = cdw * sc; agg[3, rel] += cdt^T @ oh in psum.
  - PSUM: zp ring (z1p/z2p [128, RUNW] f32) x2 bufs, z3p [128, 2TP] x2,
    aggp [3, BLKR] x2 -> 8 banks when TP<=8.
  - Gathers keep the xbar-flush guard: z1B mms of run r-2 wait on the
    gather instruction of run r (same-queue reuse distance).
  - Output: per-block (agg + x)*flags -> outT [3, NB*BLKR]; concat cores,
    transpose, trim to [50000, 3].
"""
import sys
import os

sys.path.insert(0, "/opt/trn_rl_repo")

import numpy as np
import ml_dtypes

N_NODES = 50000
N_EDGES = 800000
HIDDEN = 128
EDGE_DIM = 16
COORDS_RANGE = 15.0
NCORES = 8
P = 128
NPC = N_NODES // NCORES          # 6250 nodes per core
BLKR = 112                       # rows per node block
NB = (NPC + BLKR - 1) // BLKR    # 56 blocks per core
NPAD = NB * BLKR                 # 6272 padded nodes per core
C0 = 25000                       # gather table split (int16 idx range)
RCH = 4                          # runs per input chunk (even: 2 blocks)

_BF16 = ml_dtypes.bfloat16
_FP8 = ml_dtypes.float8_e4m3
SINGLE_PACKET = os.environ.get("SP", "1") == "1"


def _wrap_idx(idx_call):
    """int16 index list [NI] -> [128, NI//16] (16-part wrap, replicated 8x)."""
    ni = idx_call.shape[0]
    w = idx_call.reshape(ni // 16, 16).T  # [16, NI//16]
    return np.tile(w, (8, 1))             # [128, NI//16]


def _call_sizes(RUNW):
    """Split RUNW into gather-call sizes: multiples of 128, starts at 512
    multiples (so z1 psum chunks never straddle a call), each <= 896."""
    k, rem = RUNW // 512, RUNW % 512
    if k == 0:
        return [RUNW]
    if rem and 512 + rem <= 896:
        return [512] * (k - 1) + [512 + rem]
    return [512] * k + ([rem] if rem else [])


def _build_nc(TP, call_nis):
    import concourse.bass as bass
    import concourse.mybir as mybir
    import concourse.tile as tile
    from concourse import bacc
    from concourse import library_config

    dt = mybir.dt
    RUNW = TP * P                    # edge slots per run
    NRUNS = NB * 2
    S = NRUNS * RUNW                 # edge slots per core
    OHW = NRUNS * TP * BLKR          # oh dram cols
    NCH = (NRUNS + RCH - 1) // RCH   # input chunks
    ED1 = EDGE_DIM + 1

    nc = bacc.Bacc("TRN2", target_bir_lowering=False, debug=False,
                   num_devices=NCORES, num_swdge_queues=4,
                   dynamic_dma_scratch_size=65536)

    hlo = nc.dram_tensor("hlo", [C0 + P, HIDDEN], dt.bfloat16, kind="ExternalInput")
    hhi = nc.dram_tensor("hhi", [N_NODES - C0 + P, HIDDEN], dt.bfloat16, kind="ExternalInput")
    idxw = nc.dram_tensor("idxw", [P, S // 16], dt.int16, kind="ExternalInput")
    Mh = nc.dram_tensor("Mh", [BLKR, S], dt.float8e4, kind="ExternalInput")
    ohh = nc.dram_tensor("ohh", [P, OHW], dt.float8e4, kind="ExternalInput")
    eaT = nc.dram_tensor("eaT", [ED1, S], dt.bfloat16, kind="ExternalInput")
    cdw = nc.dram_tensor("cdw", [P, NRUNS * TP * 3], dt.bfloat16, kind="ExternalInput")
    hTs = nc.dram_tensor("hTs", [P, NPAD], dt.bfloat16, kind="ExternalInput")
    xT3 = nc.dram_tensor("xT3", [3, NPAD], dt.float32, kind="ExternalInput")
    flg3 = nc.dram_tensor("flg3", [3, NPAD], dt.float32, kind="ExternalInput")
    w1a = nc.dram_tensor("w1a", [HIDDEN, HIDDEN], dt.bfloat16, kind="ExternalInput")
    w1b = nc.dram_tensor("w1b", [HIDDEN, HIDDEN], dt.bfloat16, kind="ExternalInput")
    w1c = nc.dram_tensor("w1c", [ED1, HIDDEN], dt.bfloat16, kind="ExternalInput")
    w2 = nc.dram_tensor("w2", [HIDDEN, HIDDEN], dt.bfloat16, kind="ExternalInput")
    w3 = nc.dram_tensor("w3", [HIDDEN, 1], dt.bfloat16, kind="ExternalInput")
    b2 = nc.dram_tensor("b2", [HIDDEN, 1], dt.float32, kind="ExternalInput")
    outT = nc.dram_tensor("outT", [3, NPAD], dt.float32, kind="ExternalOutput")

    AF = mybir.ActivationFunctionType
    ALU = mybir.AluOpType

    # PSUM: 8 banks. z1p/z2p ring = 2 * ceil(RUNW*4/2048) banks; z3p/aggp
    # rings fill the rest (2+2 when TP<=8, 1+1 at TP=9).
    zp_banks = -(-RUNW * 4 // 2048)
    small_bufs = 2 if 2 * zp_banks + 4 <= 8 else 1

    with tile.TileContext(nc) as tc:
        nc.gpsimd.load_library(library_config.mlp)
        tc.strict_bb_all_engine_barrier()
        with (
            tc.tile_pool(name="const", bufs=1) as cp,
            tc.tile_pool(name="gath", bufs=8) as gp,
            tc.tile_pool(name="chunk", bufs=2) as chp,
            tc.tile_pool(name="work", bufs=2) as wp,
            tc.tile_pool(name="small", bufs=2) as scp,
            tc.tile_pool(name="zp", bufs=2, space="PSUM") as zp,
            tc.tile_pool(name="zq", bufs=small_bufs, space="PSUM") as zq,
            tc.tile_pool(name="pagg", bufs=small_bufs, space="PSUM") as pa,
        ):
            # ---- resident constants
            w1a_sb = cp.tile([HIDDEN, HIDDEN], dt.bfloat16)
            nc.sync.dma_start(out=w1a_sb[:], in_=w1a[:])
            w1b_sb = cp.tile([HIDDEN, HIDDEN], dt.bfloat16)
            nc.sync.dma_start(out=w1b_sb[:], in_=w1b[:])
            w1c_sb = cp.tile([ED1, HIDDEN], dt.bfloat16)
            nc.sync.dma_start(out=w1c_sb[:], in_=w1c[:])
            w2_sb = cp.tile([HIDDEN, HIDDEN], dt.bfloat16)
            nc.sync.dma_start(out=w2_sb[:], in_=w2[:])
            w3_sb = cp.tile([HIDDEN, 1], dt.bfloat16)
            nc.sync.dma_start(out=w3_sb[:], in_=w3[:])
            b2_sb = cp.tile([HIDDEN, 1], dt.float32)
            nc.sync.dma_start(out=b2_sb[:], in_=b2[:])
            idx_sb = cp.tile([P, S // 16], dt.int16)
            nc.sync.dma_start(out=idx_sb[:], in_=idxw[:])
            cdw_sb = cp.tile([P, NB, 2 * TP, 3], dt.bfloat16)
            nc.sync.dma_start(
                out=cdw_sb[:],
                in_=cdw[:].rearrange("p (b t c) -> p b t c", b=NB, t=2 * TP))
            hTs_sb = cp.tile([P, NPAD], dt.bfloat16)
            nc.sync.dma_start(out=hTs_sb[:], in_=hTs[:])
            x_sb = cp.tile([3, NPAD], dt.float32)
            nc.sync.dma_start(out=x_sb[:], in_=xT3[:])
            f_sb = cp.tile([3, NPAD], dt.float32)
            nc.sync.dma_start(out=f_sb[:], in_=flg3[:])

            # ---- q = h @ W1a per node block, node-major [rel, feat]
            q_sb = cp.tile([BLKR, NB, HIDDEN], dt.bfloat16)
            for b in range(NB):
                qp = zp.tile([BLKR, HIDDEN], dt.float32, tag="zp")
                nc.tensor.matmul(qp[:], lhsT=hTs_sb[:, b * BLKR:(b + 1) * BLKR],
                                 rhs=w1a_sb[:], start=True, stop=True)
                nc.vector.tensor_copy(out=q_sb[:, b, :], in_=qp[:])
            tc.strict_bb_all_engine_barrier()

            # ---- chunked inputs (M, oh, ea) with 1-chunk lookahead
            chunks = {}

            def fetch_chunk(k):
                if k >= NCH or k in chunks:
                    return
                mch = chp.tile([BLKR, RCH * RUNW], dt.float8e4, tag="M")
                nc.sync.dma_start(out=mch[:], in_=Mh[:, k * RCH * RUNW:(k + 1) * RCH * RUNW])
                ohch = chp.tile([P, RCH * TP * BLKR], dt.float8e4, tag="oh")
                nc.sync.dma_start(
                    out=ohch[:],
                    in_=ohh[:, k * RCH * TP * BLKR:(k + 1) * RCH * TP * BLKR])
                each = chp.tile([ED1, RCH * RUNW], dt.bfloat16, tag="ea")
                nc.sync.dma_start(out=each[:], in_=eaT[:, k * RCH * RUNW:(k + 1) * RCH * RUNW])
                chunks[k] = (mch, ohch, each)

            fetch_chunk(0)
            fetch_chunk(1)

            # ---- main loop
            from concourse.bass import _add_dep_helper
            z1b_by_run = {}
            call_off = [0]
            for ni in call_nis:
                call_off.append(call_off[-1] + ni)
            gcall = 0
            for b in range(NB):
                z3p = zq.tile([P, 2 * TP], dt.float32, tag="z3")
                for r in range(2):
                    run = b * 2 + r
                    k = run // RCH
                    if run % RCH == 0:
                        fetch_chunk(k + 1)
                    mch, ohch, each = chunks[k]
                    roff = (run - k * RCH) * RUNW          # run offset in chunk
                    e0 = run * RUNW                        # first slot of run
                    htab = hlo if r == 0 else hhi

                    # col gathers for the run
                    hcs = []
                    for ci, ni in enumerate(call_nis):
                        hc = gp.tile([P, 1, ni], dt.bfloat16, tag=f"hc{ci}")
                        gi = nc.gpsimd.dma_gather(
                            hc[:], htab[:],
                            idx_sb[:, (e0 + call_off[ci]) // 16:(e0 + call_off[ci + 1]) // 16],
                            ni, ni, HIDDEN, transpose=True,
                            queue_num=gcall % 4, single_packet=SINGLE_PACKET,
                        )
                        gcall += 1
                        # xbar-flush guard: consumers of the gather issued two
                        # runs earlier wait until this gather retired on Q7.
                        for prev in z1b_by_run.get(run - 2, ()):
                            _add_dep_helper(prev, gi.ins,
                                            reason="gather xbar-flush guard")
                        hcs.append(hc)

                    z1p = zp.tile([P, RUNW], dt.float32, tag="zp")
                    z1b_list = []
                    for c0 in range(0, RUNW, 512):
                        cw = min(512, RUNW - c0)
                        # locate gather call containing [c0, c0+cw)
                        ci = next(i for i in range(len(call_nis))
                                  if call_off[i] <= c0 and c0 + cw <= call_off[i + 1])
                        mm = nc.tensor.matmul(
                            z1p[:, c0:c0 + cw], lhsT=w1b_sb[:],
                            rhs=hcs[ci][:, 0, c0 - call_off[ci]:c0 - call_off[ci] + cw],
                            start=True, stop=False)
                        z1b_list.append(mm.ins)
                        nc.tensor.matmul(
                            z1p[:, c0:c0 + cw], lhsT=w1c_sb[:],
                            rhs=each[:, roff + c0:roff + c0 + cw],
                            start=False, stop=False)
                        nc.tensor.matmul(
                            z1p[:, c0:c0 + cw], lhsT=q_sb[:, b, :],
                            rhs=mch[:, roff + c0:roff + c0 + cw],
                            start=False, stop=True)
                    z1b_by_run[run] = z1b_list

                    z1sb = wp.tile([P, RUNW], dt.bfloat16, tag="z1")
                    nc.scalar.activation(out=z1sb[:], in_=z1p[:], func=AF.Silu)
                    z2p = zp.tile([P, RUNW], dt.float32, tag="zp")
                    for c0 in range(0, RUNW, 512):
                        cw = min(512, RUNW - c0)
                        nc.tensor.matmul(z2p[:, c0:c0 + cw], lhsT=w2_sb[:],
                                         rhs=z1sb[:, c0:c0 + cw], start=True, stop=True)
                    z2sb = wp.tile([P, RUNW], dt.bfloat16, tag="z2")
                    nc.scalar.activation(out=z2sb[:], in_=z2p[:], func=AF.Silu,
                                         bias=b2_sb[:])
                    for t in range(TP):
                        el = t * P
                        nc.tensor.matmul(z3p[:, r * TP + t:r * TP + t + 1],
                                         lhsT=z2sb[:, el:el + P], rhs=w3_sb[:],
                                         start=True, stop=True)

                # ---- block epilogue (after both runs)
                sc = scp.tile([P, 2 * TP], dt.bfloat16, tag="sc")
                nc.scalar.activation(out=sc[:], in_=z3p[:], func=AF.Tanh)
                cdt = scp.tile([P, 2 * TP, 3], dt.bfloat16, tag="cdt")
                nc.vector.tensor_tensor(
                    out=cdt[:], in0=cdw_sb[:, b, :, :],
                    in1=sc[:].to_broadcast([P, 2 * TP, 3]), op=ALU.mult)
                aggp = pa.tile([3, BLKR], dt.float32, tag="agg")
                kb = (2 * b) // RCH
                ohc = chunks[kb][1]
                ooff = (2 * b - kb * RCH) * TP * BLKR
                for t in range(2 * TP):
                    nc.tensor.matmul(
                        aggp[:], lhsT=cdt[:, t, :],
                        rhs=ohc[:, ooff + t * BLKR:ooff + (t + 1) * BLKR],
                        start=(t == 0), stop=(t == 2 * TP - 1))
                osb = scp.tile([3, BLKR], dt.float32, tag="osb")
                nc.vector.tensor_tensor(out=osb[:], in0=aggp[:],
                                        in1=x_sb[:, b * BLKR:(b + 1) * BLKR], op=ALU.add)
                nc.vector.tensor_tensor(out=osb[:], in0=osb[:],
                                        in1=f_sb[:, b * BLKR:(b + 1) * BLKR], op=ALU.mult)
                nc.sync.dma_start(out=outT[:, b * BLKR:(b + 1) * BLKR], in_=osb[:])
                # free chunks fully consumed (keep dict small)
                done = (2 * b + 2) // RCH - 1
                chunks.pop(done - 1, None)
    nc.compile()
    return nc


def _host_prep(h, x, edge_index, edge_attr, coord_diff, flags):
    """Sort/group/pad edges; build per-core input maps.
    Returns (in_maps, TP, call_nis)."""
    row = np.asarray(edge_index[0], dtype=np.int64)
    col = np.asarray(edge_index[1], dtype=np.int64)
    E = row.shape[0]

    core = row // NPC
    rl = row % NPC
    blk = rl // BLKR                        # 0..NB-1
    rel = (rl - blk * BLKR).astype(np.int16)  # 0..BLKR-1
    half = (col >= C0).astype(np.int64)
    key = (core * NB + blk) * 2 + half
    order = np.argsort(key, kind="stable")
    ksort = key[order]
    ngroups = NCORES * NB * 2
    counts = np.bincount(ksort, minlength=ngroups)
    TP = max(int((counts.max() + P - 1) // P), 1)
    RUNW = TP * P
    NRUNS = NB * 2
    S = NRUNS * RUNW
    OHW = NRUNS * TP * BLKR
    call_nis = _call_sizes(RUNW)

    gstart = np.zeros(ngroups + 1, dtype=np.int64)
    gstart[1:] = np.cumsum(counts)
    within = np.arange(E, dtype=np.int64) - gstart[ksort]
    glocal = ksort % NRUNS
    slot = glocal * RUNW + within            # slot on the core
    ecore = ksort // NRUNS

    h_bf = np.ascontiguousarray(np.asarray(h, np.float32).astype(_BF16))
    hlo = np.zeros((C0 + P, HIDDEN), dtype=_BF16)
    hlo[:C0] = h_bf[:C0]
    hhi = np.zeros((N_NODES - C0 + P, HIDDEN), dtype=_BF16)
    hhi[:N_NODES - C0] = h_bf[C0:]
    hT = np.ascontiguousarray(h_bf.T)        # [128, N]

    ea = np.asarray(edge_attr, np.float32)
    cd15 = (np.asarray(coord_diff, np.float32) * COORDS_RANGE).astype(_BF16)

    in_maps = []
    for c in range(NCORES):
        m = ecore == c
        sl = slot[m]
        eidx = order[m]
        relc = rel[eidx]
        tix = sl // P % TP                    # tile within run
        pix = sl % P                          # partition (edge in tile)
        runc = sl // RUNW                     # run index

        idx = np.zeros(S, dtype=np.int16)
        idx[sl] = (col[eidx] - half[eidx] * C0).astype(np.int16)
        Mm = np.zeros((BLKR, S), dtype=_FP8)
        Mm[relc, sl] = np.float32(1.0)
        oh = np.zeros((P, OHW), dtype=_FP8)
        oh[pix, (runc * TP + tix) * BLKR + relc] = np.float32(1.0)
        eaTc = np.zeros((EDGE_DIM + 1, S), dtype=_BF16)
        eaTc[:EDGE_DIM, sl] = ea[eidx].T.astype(_BF16)
        eaTc[EDGE_DIM, sl] = np.float32(1.0)
        cdwc = np.zeros((P, NRUNS * TP * 3), dtype=_BF16)
        cdwc[pix, (runc * TP + tix) * 3 + 0] = cd15[eidx, 0]
        cdwc[pix, (runc * TP + tix) * 3 + 1] = cd15[eidx, 1]
        cdwc[pix, (runc * TP + tix) * 3 + 2] = cd15[eidx, 2]

        idxw = np.zeros((P, S // 16), dtype=np.int16)
        coff = 0
        for g in range(NRUNS):
            base = g * RUNW
            for ni in call_nis:
                idxw[:, coff:coff + ni // 16] = _wrap_idx(idx[base:base + ni])
                base += ni
                coff += ni // 16

        n0 = c * NPC
        hTs = np.zeros((P, NPAD), dtype=_BF16)
        hTs[:, :NPC] = hT[:, n0:n0 + NPC]
        xT3 = np.zeros((3, NPAD), dtype=np.float32)
        xT3[:, :NPC] = np.asarray(x, np.float32)[n0:n0 + NPC].T
        flg3 = np.zeros((3, NPAD), dtype=np.float32)
        flg3[:, :NPC] = np.broadcast_to(
            np.asarray(flags, np.float32)[n0:n0 + NPC].T, (3, NPC))

        in_maps.append({
            "hlo": hlo, "hhi": hhi, "idxw": idxw, "Mh": Mm, "ohh": oh,
            "eaT": np.ascontiguousarray(eaTc), "cdw": cdwc,
            "hTs": hTs, "xT3": xT3, "flg3": flg3,
        })
    return in_maps, TP, call_nis


def kernel(h, x, edge_index, edge_attr, coord_diff, flags, edge_mask,
           W1, b1, W2, b2, W3):
    from concourse.bass_utils import run_bass_kernel_spmd

    h = np.asarray(h, dtype=np.float32)
    x = np.asarray(x, dtype=np.float32)
    in_maps, TP, call_nis = _host_prep(
        h, x, np.asarray(edge_index), np.asarray(edge_attr),
        np.asarray(coord_diff), np.asarray(flags))

    W1 = np.asarray(W1, dtype=np.float32)
    w1c = np.zeros((EDGE_DIM + 1, HIDDEN), dtype=_BF16)
    w1c[:EDGE_DIM] = W1[2 * HIDDEN:].astype(_BF16)
    w1c[EDGE_DIM] = np.asarray(b1, dtype=np.float32).astype(_BF16)
    wshare = {
        "w1a": np.ascontiguousarray(W1[:HIDDEN].astype(_BF16)),
        "w1b": np.ascontiguousarray(W1[HIDDEN:2 * HIDDEN].astype(_BF16)),
        "w1c": w1c,
        "w2": np.ascontiguousarray(np.asarray(W2, np.float32).astype(_BF16)),
        "w3": np.ascontiguousarray(np.asarray(W3, np.float32).astype(_BF16)),
        "b2": np.asarray(b2, np.float32).reshape(HIDDEN, 1),
    }
    for m in in_maps:
        m.update(wshare)

    nc = _build_nc(TP, call_nis)
    res = run_bass_kernel_spmd(nc, in_maps, core_ids=list(range(NCORES)),
                               trace=os.environ.get("BASS_TRACE") == "1")
    global last_result
    last_result = res
    out = np.empty((N_NODES, 3), dtype=np.float32)
    for c in range(NCORES):
        out[c * NPC:(c + 1) * NPC] = res.results[c]["outT"][:, :NPC].T
    return out


last_result = None


# revision 9
# speedup vs baseline: 2.1221x; 1.3403x over previous
"""E3CoordLayer GNN message-passing kernel for 8 Trainium2 NeuronCores.

Strategy (edge-parallel, row-range sharded, v2):
  - Sort edges by row; core c owns rows [c*6250, (c+1)*6250). Rows grouped
    into NB blocks of BLKR=112 rows; within a block, edges split into 2 runs
    by col range (col < 25000 -> lo table, else hi table) so the h gather
    uses 256B single-row descriptors with int16 indices.
  - Each run is padded to RUNW = TP*128 slots (TP = global max, uniform for
    SPMD). Gathers use single_packet=False so descriptors drain across all
    16 SDMA engines instead of one engine per call.
  - h[row] is never gathered: q = h @ W1a computed per node block on device;
    the per-edge expansion q[row_e] uses a host-shipped fp8 one-hot
    M[rel, slot] as the matmul rhs (no on-device M build).
  - The aggregation one-hot oh[slot%128, (t, rel)] is also host-shipped fp8.
  - MLP runs feature-major: z1[h1,e] accumulates W1b^T hcol + W1c'^T ea
    (b1 folded via a ones-row in eaT) + q^T M; silu; z2 = W2^T z1sb; silu
    with b2 bias; z3 per tile via lhsT=z2-tile, rhs=w3 -> z3p[p, r*TP+t];
    tanh once per block; cdt = cdw * sc; agg[3, rel] += cdt^T @ oh in psum.
  - PSUM: zp ring (z1p/z2p [128, RUNW] f32) x2 bufs, z3p [128, 2TP] x2,
    aggp [3, BLKR] x2 -> 8 banks when TP<=8.
  - Gathers keep the xbar-flush guard: z1B mms of run r-2 wait on the
    gather instruction of run r (same-queue reuse distance).
  - Output: per-block (agg + x)*flags -> outT [3, NB*BLKR]; concat cores,
    transpose, trim to [50000, 3].
"""
import sys
import os

sys.path.insert(0, "/opt/trn_rl_repo")

import numpy as np
import ml_dtypes

N_NODES = 50000
N_EDGES = 800000
HIDDEN = 128
EDGE_DIM = 16
COORDS_RANGE = 15.0
NCORES = 8
P = 128
NPC = N_NODES // NCORES          # 6250 nodes per core
BLKR = 112                       # rows per node block
NB = (NPC + BLKR - 1) // BLKR    # 56 blocks per core
NPAD = NB * BLKR                 # 6272 padded nodes per core
C0 = 25000                       # gather table split (int16 idx range)
RCH = 4                          # runs per input chunk (even: 2 blocks)

_BF16 = ml_dtypes.bfloat16
_FP8 = ml_dtypes.float8_e4m3
SINGLE_PACKET = os.environ.get("SP", "1") == "1"


def _wrap_idx(idx_call):
    """int16 index list [NI] -> [128, NI//16] (16-part wrap, replicated 8x)."""
    ni = idx_call.shape[0]
    w = idx_call.reshape(ni // 16, 16).T  # [16, NI//16]
    return np.tile(w, (8, 1))             # [128, NI//16]


def _call_sizes(RUNW):
    """Split RUNW into gather-call sizes: multiples of 128, starts at 512
    multiples (so z1 psum chunks never straddle a call), each <= 896."""
    k, rem = RUNW // 512, RUNW % 512
    if k == 0:
        return [RUNW]
    if rem and 512 + rem <= 896:
        return [512] * (k - 1) + [512 + rem]
    return [512] * k + ([rem] if rem else [])


def _build_nc(TP, call_nis):
    import concourse.bass as bass
    import concourse.mybir as mybir
    import concourse.tile as tile
    from concourse import bacc
    from concourse import library_config

    dt = mybir.dt
    RUNW = TP * P                    # edge slots per run
    NRUNS = NB * 2
    S = NRUNS * RUNW                 # edge slots per core
    OHW = NRUNS * TP * BLKR          # oh dram cols
    NCH = (NRUNS + RCH - 1) // RCH   # input chunks
    ED1 = EDGE_DIM + 1

    nc = bacc.Bacc("TRN2", target_bir_lowering=False, debug=False,
                   num_devices=NCORES, num_swdge_queues=4,
                   dynamic_dma_scratch_size=65536)

    hlo = nc.dram_tensor("hlo", [C0 + P, HIDDEN], dt.bfloat16, kind="ExternalInput")
    hhi = nc.dram_tensor("hhi", [N_NODES - C0 + P, HIDDEN], dt.bfloat16, kind="ExternalInput")
    idxw = nc.dram_tensor("idxw", [P, S // 16], dt.int16, kind="ExternalInput")
    Mh = nc.dram_tensor("Mh", [BLKR, S], dt.float8e4, kind="ExternalInput")
    ohh = nc.dram_tensor("ohh", [P, OHW], dt.float8e4, kind="ExternalInput")
    eaT = nc.dram_tensor("eaT", [ED1, S], dt.bfloat16, kind="ExternalInput")
    cdw = nc.dram_tensor("cdw", [P, NRUNS * TP * 3], dt.bfloat16, kind="ExternalInput")
    hTs = nc.dram_tensor("hTs", [P, NPAD], dt.bfloat16, kind="ExternalInput")
    xT3 = nc.dram_tensor("xT3", [3, NPAD], dt.float32, kind="ExternalInput")
    flg3 = nc.dram_tensor("flg3", [3, NPAD], dt.float32, kind="ExternalInput")
    w1a = nc.dram_tensor("w1a", [HIDDEN, HIDDEN], dt.bfloat16, kind="ExternalInput")
    w1b = nc.dram_tensor("w1b", [HIDDEN, HIDDEN], dt.bfloat16, kind="ExternalInput")
    w1c = nc.dram_tensor("w1c", [ED1, HIDDEN], dt.bfloat16, kind="ExternalInput")
    w2 = nc.dram_tensor("w2", [HIDDEN, HIDDEN], dt.bfloat16, kind="ExternalInput")
    w3 = nc.dram_tensor("w3", [HIDDEN, 1], dt.bfloat16, kind="ExternalInput")
    b2 = nc.dram_tensor("b2", [HIDDEN, 1], dt.float32, kind="ExternalInput")
    outT = nc.dram_tensor("outT", [3, NPAD], dt.float32, kind="ExternalOutput")

    AF = mybir.ActivationFunctionType
    ALU = mybir.AluOpType

    # PSUM: 8 banks. z1p/z2p ring wants 3 bufs (so next-run z1 matmuls can
    # start while this run's silus drain) + 1 bank each for z3p/aggp.
    zp_banks = -(-RUNW * 4 // 2048)
    zp_bufs = 3 if 3 * zp_banks + 2 <= 8 else 2
    small_bufs = 1

    with tile.TileContext(nc) as tc:
        nc.gpsimd.load_library(library_config.mlp)
        tc.strict_bb_all_engine_barrier()
        with (
            tc.tile_pool(name="const", bufs=1) as cp,
            tc.tile_pool(name="gath", bufs=8) as gp,
            tc.tile_pool(name="chunk", bufs=2) as chp,
            tc.tile_pool(name="work", bufs=2) as wp,
            tc.tile_pool(name="small", bufs=2) as scp,
            tc.tile_pool(name="zp", bufs=zp_bufs, space="PSUM") as zp,
            tc.tile_pool(name="zq", bufs=small_bufs, space="PSUM") as zq,
            tc.tile_pool(name="pagg", bufs=small_bufs, space="PSUM") as pa,
        ):
            # ---- resident constants
            w1a_sb = cp.tile([HIDDEN, HIDDEN], dt.bfloat16)
            nc.sync.dma_start(out=w1a_sb[:], in_=w1a[:])
            w1b_sb = cp.tile([HIDDEN, HIDDEN], dt.bfloat16)
            nc.sync.dma_start(out=w1b_sb[:], in_=w1b[:])
            w1c_sb = cp.tile([ED1, HIDDEN], dt.bfloat16)
            nc.sync.dma_start(out=w1c_sb[:], in_=w1c[:])
            w2_sb = cp.tile([HIDDEN, HIDDEN], dt.bfloat16)
            nc.sync.dma_start(out=w2_sb[:], in_=w2[:])
            w3_sb = cp.tile([HIDDEN, 1], dt.bfloat16)
            nc.sync.dma_start(out=w3_sb[:], in_=w3[:])
            b2_sb = cp.tile([HIDDEN, 1], dt.float32)
            nc.sync.dma_start(out=b2_sb[:], in_=b2[:])
            idx_sb = cp.tile([P, S // 16], dt.int16)
            nc.sync.dma_start(out=idx_sb[:], in_=idxw[:])
            cdw_sb = cp.tile([P, NB, 2 * TP, 3], dt.bfloat16)
            nc.sync.dma_start(
                out=cdw_sb[:],
                in_=cdw[:].rearrange("p (b t c) -> p b t c", b=NB, t=2 * TP))
            hTs_sb = cp.tile([P, NPAD], dt.bfloat16)
            nc.sync.dma_start(out=hTs_sb[:], in_=hTs[:])
            x_sb = cp.tile([3, NPAD], dt.float32)
            nc.sync.dma_start(out=x_sb[:], in_=xT3[:])
            f_sb = cp.tile([3, NPAD], dt.float32)
            nc.sync.dma_start(out=f_sb[:], in_=flg3[:])

            # ---- q = h @ W1a per node block, node-major [rel, feat]
            q_sb = cp.tile([BLKR, NB, HIDDEN], dt.bfloat16)
            for b in range(NB):
                qp = zp.tile([BLKR, HIDDEN], dt.float32, tag="zp")
                nc.tensor.matmul(qp[:], lhsT=hTs_sb[:, b * BLKR:(b + 1) * BLKR],
                                 rhs=w1a_sb[:], start=True, stop=True)
                nc.vector.tensor_copy(out=q_sb[:, b, :], in_=qp[:])
            tc.strict_bb_all_engine_barrier()

            # ---- chunked inputs (M, oh, ea) with 1-chunk lookahead
            chunks = {}

            def fetch_chunk(k):
                if k >= NCH or k in chunks:
                    return
                mch = chp.tile([BLKR, RCH * RUNW], dt.float8e4, tag="M")
                nc.sync.dma_start(out=mch[:], in_=Mh[:, k * RCH * RUNW:(k + 1) * RCH * RUNW])
                ohch = chp.tile([P, RCH * TP * BLKR], dt.float8e4, tag="oh")
                nc.sync.dma_start(
                    out=ohch[:],
                    in_=ohh[:, k * RCH * TP * BLKR:(k + 1) * RCH * TP * BLKR])
                each = chp.tile([ED1, RCH * RUNW], dt.bfloat16, tag="ea")
                nc.sync.dma_start(out=each[:], in_=eaT[:, k * RCH * RUNW:(k + 1) * RCH * RUNW])
                chunks[k] = (mch, ohch, each)

            fetch_chunk(0)
            fetch_chunk(1)

            # ---- main loop
            from concourse.bass import _add_dep_helper
            z1b_by_run = {}
            call_off = [0]
            for ni in call_nis:
                call_off.append(call_off[-1] + ni)
            gcall = 0
            for b in range(NB):
                z3p = zq.tile([P, 2 * TP], dt.float32, tag="z3")
                for r in range(2):
                    run = b * 2 + r
                    k = run // RCH
                    if run % RCH == 0:
                        fetch_chunk(k + 1)
                    mch, ohch, each = chunks[k]
                    roff = (run - k * RCH) * RUNW          # run offset in chunk
                    e0 = run * RUNW                        # first slot of run
                    htab = hlo if r == 0 else hhi

                    # col gathers for the run
                    hcs = []
                    for ci, ni in enumerate(call_nis):
                        hc = gp.tile([P, 1, ni], dt.bfloat16, tag=f"hc{ci}")
                        gi = nc.gpsimd.dma_gather(
                            hc[:], htab[:],
                            idx_sb[:, (e0 + call_off[ci]) // 16:(e0 + call_off[ci + 1]) // 16],
                            ni, ni, HIDDEN, transpose=True,
                            queue_num=gcall % 4, single_packet=SINGLE_PACKET,
                        )
                        gcall += 1
                        # xbar-flush guard: consumers of the gather issued one
                        # run earlier wait until this gather retired on Q7
                        # (gives the transposed DMA writes time to land).
                        for prev in z1b_by_run.get(run - 1, ()):
                            _add_dep_helper(prev, gi.ins,
                                            reason="gather xbar-flush guard")
                        hcs.append(hc)

                    z1p = zp.tile([P, RUNW], dt.float32, tag="zp")
                    # grouped by weight so LDWEIGHTS is paid once per weight,
                    # not once per (weight, chunk)
                    z1b_list = []
                    for c0 in range(0, RUNW, 512):
                        cw = min(512, RUNW - c0)
                        # locate gather call containing [c0, c0+cw)
                        ci = next(i for i in range(len(call_nis))
                                  if call_off[i] <= c0 and c0 + cw <= call_off[i + 1])
                        mm = nc.tensor.matmul(
                            z1p[:, c0:c0 + cw], lhsT=w1b_sb[:],
                            rhs=hcs[ci][:, 0, c0 - call_off[ci]:c0 - call_off[ci] + cw],
                            start=True, stop=False)
                        z1b_list.append(mm.ins)
                    for c0 in range(0, RUNW, 512):
                        cw = min(512, RUNW - c0)
                        nc.tensor.matmul(
                            z1p[:, c0:c0 + cw], lhsT=w1c_sb[:],
                            rhs=each[:, roff + c0:roff + c0 + cw],
                            start=False, stop=False)
                    for c0 in range(0, RUNW, 512):
                        cw = min(512, RUNW - c0)
                        nc.tensor.matmul(
                            z1p[:, c0:c0 + cw], lhsT=q_sb[:, b, :],
                            rhs=mch[:, roff + c0:roff + c0 + cw],
                            start=False, stop=True)
                    z1b_by_run[run] = z1b_list

                    z1sb = wp.tile([P, RUNW], dt.bfloat16, tag="z1")
                    nc.scalar.activation(out=z1sb[:], in_=z1p[:], func=AF.Silu)
                    z2p = zp.tile([P, RUNW], dt.float32, tag="zp")
                    for c0 in range(0, RUNW, 512):
                        cw = min(512, RUNW - c0)
                        nc.tensor.matmul(z2p[:, c0:c0 + cw], lhsT=w2_sb[:],
                                         rhs=z1sb[:, c0:c0 + cw], start=True, stop=True)
                    z2sb = wp.tile([P, RUNW], dt.bfloat16, tag="z2")
                    nc.scalar.activation(out=z2sb[:], in_=z2p[:], func=AF.Silu,
                                         bias=b2_sb[:])
                    for t in range(TP):
                        el = t * P
                        nc.tensor.matmul(z3p[:, r * TP + t:r * TP + t + 1],
                                         lhsT=z2sb[:, el:el + P], rhs=w3_sb[:],
                                         start=True, stop=True)

                # ---- block epilogue (after both runs)
                sc = scp.tile([P, 2 * TP], dt.bfloat16, tag="sc")
                nc.scalar.activation(out=sc[:], in_=z3p[:], func=AF.Tanh)
                cdt = scp.tile([P, 2 * TP, 3], dt.bfloat16, tag="cdt")
                nc.vector.tensor_tensor(
                    out=cdt[:], in0=cdw_sb[:, b, :, :],
                    in1=sc[:].to_broadcast([P, 2 * TP, 3]), op=ALU.mult)
                aggp = pa.tile([3, BLKR], dt.float32, tag="agg")
                kb = (2 * b) // RCH
                ohc = chunks[kb][1]
                ooff = (2 * b - kb * RCH) * TP * BLKR
                for t in range(2 * TP):
                    nc.tensor.matmul(
                        aggp[:], lhsT=cdt[:, t, :],
                        rhs=ohc[:, ooff + t * BLKR:ooff + (t + 1) * BLKR],
                        start=(t == 0), stop=(t == 2 * TP - 1))
                osb = scp.tile([3, BLKR], dt.float32, tag="osb")
                nc.vector.tensor_tensor(out=osb[:], in0=aggp[:],
                                        in1=x_sb[:, b * BLKR:(b + 1) * BLKR], op=ALU.add)
                nc.vector.tensor_tensor(out=osb[:], in0=osb[:],
                                        in1=f_sb[:, b * BLKR:(b + 1) * BLKR], op=ALU.mult)
                nc.sync.dma_start(out=outT[:, b * BLKR:(b + 1) * BLKR], in_=osb[:])
                # free chunks fully consumed (keep dict small)
                done = (2 * b + 2) // RCH - 1
                chunks.pop(done - 1, None)
    nc.compile()
    return nc


def _host_prep(h, x, edge_index, edge_attr, coord_diff, flags):
    """Sort/group/pad edges; build per-core input maps.
    Returns (in_maps, TP, call_nis)."""
    row = np.asarray(edge_index[0], dtype=np.int64)
    col = np.asarray(edge_index[1], dtype=np.int64)
    E = row.shape[0]

    core = row // NPC
    rl = row % NPC
    blk = rl // BLKR                        # 0..NB-1
    rel = (rl - blk * BLKR).astype(np.int16)  # 0..BLKR-1
    half = (col >= C0).astype(np.int64)
    key = (core * NB + blk) * 2 + half
    order = np.argsort(key, kind="stable")
    ksort = key[order]
    ngroups = NCORES * NB * 2
    counts = np.bincount(ksort, minlength=ngroups)
    TP = max(int((counts.max() + P - 1) // P), 1)
    RUNW = TP * P
    NRUNS = NB * 2
    S = NRUNS * RUNW
    OHW = NRUNS * TP * BLKR
    call_nis = _call_sizes(RUNW)

    gstart = np.zeros(ngroups + 1, dtype=np.int64)
    gstart[1:] = np.cumsum(counts)
    within = np.arange(E, dtype=np.int64) - gstart[ksort]
    glocal = ksort % NRUNS
    slot = glocal * RUNW + within            # slot on the core
    ecore = ksort // NRUNS

    h_bf = np.ascontiguousarray(np.asarray(h, np.float32).astype(_BF16))
    hlo = np.zeros((C0 + P, HIDDEN), dtype=_BF16)
    hlo[:C0] = h_bf[:C0]
    hhi = np.zeros((N_NODES - C0 + P, HIDDEN), dtype=_BF16)
    hhi[:N_NODES - C0] = h_bf[C0:]
    hT = np.ascontiguousarray(h_bf.T)        # [128, N]

    ea = np.asarray(edge_attr, np.float32)
    cd15 = (np.asarray(coord_diff, np.float32) * COORDS_RANGE).astype(_BF16)

    in_maps = []
    for c in range(NCORES):
        m = ecore == c
        sl = slot[m]
        eidx = order[m]
        relc = rel[eidx]
        tix = sl // P % TP                    # tile within run
        pix = sl % P                          # partition (edge in tile)
        runc = sl // RUNW                     # run index

        idx = np.zeros(S, dtype=np.int16)
        idx[sl] = (col[eidx] - half[eidx] * C0).astype(np.int16)
        Mm = np.zeros((BLKR, S), dtype=_FP8)
        Mm[relc, sl] = np.float32(1.0)
        oh = np.zeros((P, OHW), dtype=_FP8)
        oh[pix, (runc * TP + tix) * BLKR + relc] = np.float32(1.0)
        eaTc = np.zeros((EDGE_DIM + 1, S), dtype=_BF16)
        eaTc[:EDGE_DIM, sl] = ea[eidx].T.astype(_BF16)
        eaTc[EDGE_DIM, sl] = np.float32(1.0)
        cdwc = np.zeros((P, NRUNS * TP * 3), dtype=_BF16)
        cdwc[pix, (runc * TP + tix) * 3 + 0] = cd15[eidx, 0]
        cdwc[pix, (runc * TP + tix) * 3 + 1] = cd15[eidx, 1]
        cdwc[pix, (runc * TP + tix) * 3 + 2] = cd15[eidx, 2]

        idxw = np.zeros((P, S // 16), dtype=np.int16)
        coff = 0
        for g in range(NRUNS):
            base = g * RUNW
            for ni in call_nis:
                idxw[:, coff:coff + ni // 16] = _wrap_idx(idx[base:base + ni])
                base += ni
                coff += ni // 16

        n0 = c * NPC
        hTs = np.zeros((P, NPAD), dtype=_BF16)
        hTs[:, :NPC] = hT[:, n0:n0 + NPC]
        xT3 = np.zeros((3, NPAD), dtype=np.float32)
        xT3[:, :NPC] = np.asarray(x, np.float32)[n0:n0 + NPC].T
        flg3 = np.zeros((3, NPAD), dtype=np.float32)
        flg3[:, :NPC] = np.broadcast_to(
            np.asarray(flags, np.float32)[n0:n0 + NPC].T, (3, NPC))

        in_maps.append({
            "hlo": hlo, "hhi": hhi, "idxw": idxw, "Mh": Mm, "ohh": oh,
            "eaT": np.ascontiguousarray(eaTc), "cdw": cdwc,
            "hTs": hTs, "xT3": xT3, "flg3": flg3,
        })
    return in_maps, TP, call_nis


def kernel(h, x, edge_index, edge_attr, coord_diff, flags, edge_mask,
           W1, b1, W2, b2, W3):
    from concourse.bass_utils import run_bass_kernel_spmd

    h = np.asarray(h, dtype=np.float32)
    x = np.asarray(x, dtype=np.float32)
    in_maps, TP, call_nis = _host_prep(
        h, x, np.asarray(edge_index), np.asarray(edge_attr),
        np.asarray(coord_diff), np.asarray(flags))

    W1 = np.asarray(W1, dtype=np.float32)
    w1c = np.zeros((EDGE_DIM + 1, HIDDEN), dtype=_BF16)
    w1c[:EDGE_DIM] = W1[2 * HIDDEN:].astype(_BF16)
    w1c[EDGE_DIM] = np.asarray(b1, dtype=np.float32).astype(_BF16)
    wshare = {
        "w1a": np.ascontiguousarray(W1[:HIDDEN].astype(_BF16)),
        "w1b": np.ascontiguousarray(W1[HIDDEN:2 * HIDDEN].astype(_BF16)),
        "w1c": w1c,
        "w2": np.ascontiguousarray(np.asarray(W2, np.float32).astype(_BF16)),
        "w3": np.ascontiguousarray(np.asarray(W3, np.float32).astype(_BF16)),
        "b2": np.asarray(b2, np.float32).reshape(HIDDEN, 1),
    }
    for m in in_maps:
        m.update(wshare)

    nc = _build_nc(TP, call_nis)
    res = run_bass_kernel_spmd(nc, in_maps, core_ids=list(range(NCORES)),
                               trace=os.environ.get("BASS_TRACE") == "1")
    global last_result
    last_result = res
    out = np.empty((N_NODES, 3), dtype=np.float32)
    for c in range(NCORES):
        out[c * NPC:(c + 1) * NPC] = res.results[c]["outT"][:, :NPC].T
    return out


last_result = None
